# revision 27
# baseline (speedup 1.0000x reference)
"""Trainium2 Bass kernel for single-head attention (B=8, N=3136, C=147, D=64).

Sharding: data-parallel over batch across 8 NeuronCores (1 batch element/core).

The device runs only the O(N^2) attention core (S = q@k^T, exp, P@V) --
>99% of the FLOPs; the O(N*C*D)/O(N*D^2) projections and epilogue
(softmax normalization, W_proj, bias, v-residual) run on the host in
fp32, which removes every on-device transpose / partition broadcast.

Per-core device program (all matmuls bf16, 1 column/cycle; the PE is the
bottleneck at ~= sum of output columns = 2*N^2/128 cycles):
  - inputs: qT2/kT2 [128, N] bf16 = q^T/k^T duplicated into both partition
    halves (host-built; q pre-scaled by 1/sqrt(D)), v_aug [128, 25, 65]
    bf16 (v + ones column, zero-padded). kT2 is split across the two
    DMA queues whose fixed preamble ends earliest (Scalar/GpSimd).
  - warm-up: dummy matmuls on scratch + one dummy exp keep the PE
    continuously busy during the input DMA so the HAM clock gate opens
    (1.2 -> 2.4 GHz) before real work, and the ACT exp table preloads.
  - per 448-wide i-chunk (3136 = 7*448, uniform) x 13 j-tile pairs:
      S^T: one K=128 matmul per j-tile on the FULL duplicated tiles;
        the duplication makes the PE compute 2*S and keeps weight loads
        FWL-eligible; the 0.5 folds into the exp affine for free.
      exp: split across TWO engines so neither gates the PE -- ACT
        (scalar) takes odd pairs + the last half pair (hardware exp,
        scale=0.5); the DVE takes even pairs via a Schraudolph bit
        trick: i16 = round(2s*0.5*128*log2(e) + (127*128 - 7.37)) in a
        single tensor_scalar (fp32 PSUM -> int16 is exact
        round-to-nearest on this silicon) whose bits ARE bf16 ~= e^s.
        EB is calibrated so the mean relative error ~= 0; the +-3%
        sawtooth is pseudo-random across j and averages out under
        softmax (rel err ~8e-3 vs the 2e-2 gate).
      PV: one K=128 matmul per j-tile accumulating into o[65, icsz]
        (one PSUM bank); row 64 (ones) gathers Z for free. PV trails
        S^T by 2 pairs (st bufs=3) so both exp engines overlap.
      st tiles keep a 512-wide inner stride so each matmul output stays
      inside one 2KB PSUM bank.
  - per chunk: one DVE copy PSUM->SBUF + DMA out on the idle GpSimd
    queue; host divides by Z, projects, and adds bias + v.

Avoided dead ends (trace-verified): f32r matmuls lower to 3-pass
fp32_mode=HIGH; row-group "concurrent" matmul pairs gain nothing (the
PE is port-bound at 1 column/cycle); fp8 q/k fails the error gate;
saturating ACT+DVE+PE together trips the P0 power downclock (~2.0GHz).
"""
import sys

for _p in ("/opt/trn_rl_repo",):
    if _p not in sys.path:
        sys.path.append(_p)

import numpy as np
import ml_dtypes
from contextlib import ExitStack

import concourse.bass as bass
import concourse.bacc as bacc
import concourse.tile as tile
from concourse import mybir
from concourse.bass_utils import run_bass_kernel_spmd

P = 128
SEQ = 3136        # N
CH = 147          # C
D = 64            # head dim
SCALE = D ** -0.5
NT = (SEQ + P - 1) // P          # 25 j-tiles (24 full + 1 of 64)
IC = 448                         # i-chunk width (3136 = 7*448)
NCHUNK = (SEQ + IC - 1) // IC    # 7 (6 full + 1 of 64)
F32 = mybir.dt.float32
BF = mybir.dt.bfloat16
I16 = mybir.dt.int16
F8 = mybir.dt.float8e4
EXP = mybir.ActivationFunctionType.Exp

# Schraudolph constants: i16 = round(s * EA + EB); bits read as bf16 give
# ~e^s * (1 + eps(frac)), EB calibrated so E[eps] ~= 0.
EA = 128.0 * 1.4426950408889634
EB = 127.0 * 128.0 - 7.37

# exp engine assignment per pair index (13 pairs): True -> DVE bit trick
DVE_PAIRS = frozenset({2, 4, 6, 8, 10})

_cache = {}


def _ichunks():
    out = []
    i0 = 0
    while i0 < SEQ:
        out.append((i0, min(IC, SEQ - i0)))
        i0 += IC
    return out


def build():
    nc = bacc.Bacc("TRN2", target_bir_lowering=False, debug=False, num_devices=8)
    qT2d = nc.declare_dram_parameter("qT2", [P, SEQ], BF, isOutput=False)
    kT2d = nc.declare_dram_parameter("kT2", [P, SEQ], BF, isOutput=False)
    v_aug = nc.declare_dram_parameter("v_aug", [P, NT, D + 1], BF, isOutput=False)
    oT = nc.declare_dram_parameter("oT", [NCHUNK, D + 1, IC], F32, isOutput=True)

    with ExitStack() as ctx:
        tc = ctx.enter_context(tile.TileContext(nc))
        singles = ctx.enter_context(tc.tile_pool(name="singles", bufs=1))

        qT2 = singles.tile([P, SEQ], BF)   # qT duplicated in both halves
        kT2 = singles.tile([P, SEQ], BF)   # kT duplicated in both halves
        va = singles.tile([P, NT, D + 1], BF)
        # Issue input DMAs from the queues whose fixed preamble ends
        # earliest (Scalar/GpSimd), k first: the first S^T needs ALL of k
        # but only the first q chunk. Sync's preamble is ~2us longer, so it
        # only carries the later q chunks.
        KH = 1024            # kT2 front piece: j-tiles 0-7
        nc.scalar.dma_start(out=kT2[:, 0:KH], in_=kT2d[:, 0:KH])
        nc.gpsimd.dma_start(out=va, in_=v_aug[:, :, :])
        nc.gpsimd.dma_start(out=kT2[:, KH:SEQ], in_=kT2d[:, KH:SEQ])
        for (n0, csz) in _ichunks():
            nc.sync.dma_start(out=qT2[:, n0:n0 + csz],
                              in_=qT2d[:, n0:n0 + csz])

        # --- HAM pre-warm + ACT exp-table preload, overlapping input DMA:
        # dummy matmuls/activation on initialized scratch keep the PE
        # continuously busy so the clock gate opens (2.4 GHz) before the
        # first real matmul; results are never read.
        with ExitStack() as wctx:
            warm_ps = wctx.enter_context(
                tc.tile_pool(name="warm_ps", bufs=2, space="PSUM"))
            junk_w = singles.tile([P, P], BF)
            junk_x = singles.tile([P, IC], BF)
            junk_e = singles.tile([P, 8], F32)
            junk_p = singles.tile([P, 8], BF)
            nc.vector.memset(junk_w, 0.5)
            nc.vector.memset(junk_x, 0.5)
            nc.vector.memset(junk_e, 0.5)
            nc.scalar.activation(junk_p, junk_e, EXP)
            for _ in range(10):
                wp = warm_ps.tile([P, IC], F32, name="warm")
                nc.tensor.matmul(wp, junk_w, junk_x, start=True, stop=True)

        # ---------------- attention ----------------
        # S^T matmuls use the FULL duplicated kT2/qT2 (K=128): each scores
        # column is computed twice and summed by the PE, giving 2*S; the
        # 0.5 is folded into the exp affine for free. This keeps the weight
        # loads FWL-eligible (128 partitions x 128 bf16 columns).
        with ExitStack() as cctx:
            st_ps = cctx.enter_context(tc.tile_pool(name="st_ps", bufs=3, space="PSUM"))
            o_ps_pool = cctx.enter_context(tc.tile_pool(name="o_ps", bufs=2, space="PSUM"))
            p_pool = cctx.enter_context(tc.tile_pool(name="p_sb", bufs=5))
            o_sb_pool = cctx.enter_context(tc.tile_pool(name="o_sb", bufs=2))
            npairs = (NT + 1) // 2    # 13: 12 full pairs + 1 single

            def emit_pv(o_ps, p, pt, icsz):
                jtA, jtB = 2 * pt, 2 * pt + 1
                nc.tensor.matmul(o_ps, va[:, jtA, :], p[:, 0, 0:icsz],
                                 start=(jtA == 0), stop=False)
                if jtB < NT:
                    nc.tensor.matmul(o_ps, va[:, jtB, :], p[:, 1, 0:icsz],
                                     start=False, stop=False)

            def emit_last_pv(o_ps, p, icsz):
                jsz = SEQ - (NT - 1) * P   # 64
                nc.tensor.matmul(o_ps, va[0:jsz, NT - 1, :], p[0:jsz, 0, 0:icsz],
                                 start=False, stop=True)

            pending_out = None   # (o_ps, o_sb tile, chunk index, icsz)
            for ci, (i0, icsz) in enumerate(_ichunks()):
                o_ps = o_ps_pool.tile([D + 1, IC], F32, name="o")[:, 0:icsz]
                pend = []          # up to 2 trailing (p, pt) awaiting PV
                for pt in range(npairs):
                    jtA, jtB = 2 * pt, 2 * pt + 1
                    pair = jtB < NT
                    # inner stride stays 512 so each j-tile slice is
                    # PSUM-bank-aligned even when IC < 512
                    st = st_ps.tile([P, 2, 512], F32, name="st")
                    p = p_pool.tile([P, 2, IC], BF, name="p")
                    jwA = min(P, SEQ - jtA * P)
                    nc.tensor.matmul(
                        st[0:jwA, 0, 0:icsz],
                        kT2[:, jtA * P:jtA * P + jwA],
                        qT2[:, i0:i0 + icsz],
                        start=True, stop=True)
                    if pair:
                        nc.tensor.matmul(
                            st[:, 1, 0:icsz],
                            kT2[:, jtB * P:(jtB + 1) * P],
                            qT2[:, i0:i0 + icsz],
                            start=True, stop=True)
                        if pt in DVE_PAIRS:
                            nc.vector.tensor_scalar(
                                out=p[:, :, 0:icsz].bitcast(I16),
                                in0=st[:, :, 0:icsz],
                                scalar1=EA * 0.5, scalar2=EB,
                                op0=mybir.AluOpType.mult,
                                op1=mybir.AluOpType.add)
                        else:
                            nc.scalar.activation(p[:, :, 0:icsz], st[:, :, 0:icsz],
                                                 EXP, scale=0.5)
                    else:
                        jsz = SEQ - jtA * P
                        nc.scalar.activation(p[0:jsz, 0, 0:icsz],
                                             st[0:jsz, 0, 0:icsz], EXP, scale=0.5)
                    pend.append((p, pt))
                    if len(pend) > 2:
                        ep, ept = pend.pop(0)
                        emit_pv(o_ps, ep, ept, icsz)
                    if pt == 11 and pending_out is not None:
                        po_ps, po_sb, pci, picsz = pending_out
                        nc.vector.tensor_copy(po_sb[:, 0:picsz], po_ps)
                        nc.gpsimd.dma_start(out=oT[pci, :, 0:picsz],
                                            in_=po_sb[:, 0:picsz])
                        pending_out = None
                ep, ept = pend.pop(0)
                emit_pv(o_ps, ep, ept, icsz)
                ep, ept = pend.pop(0)
                emit_last_pv(o_ps, ep, icsz)
                o_sb = o_sb_pool.tile([D + 1, IC], F32, name="osb")
                pending_out = (o_ps, o_sb, ci, icsz)
            po_ps, po_sb, pci, picsz = pending_out
            nc.vector.tensor_copy(po_sb[:, 0:picsz], po_ps)
            nc.sync.dma_start(out=oT[pci, :, 0:picsz], in_=po_sb[:, 0:picsz])

    nc.compile()
    return nc


def prep_in_maps(x, W_qkv, W_proj, b_proj):
    """Host-side prep: per-core transposed/duplicated bf16 operand layouts."""
    B = x.shape[0]
    bf = ml_dtypes.bfloat16
    Wq = (W_qkv[:, 0:D] * SCALE).astype(np.float32)
    Wk = W_qkv[:, D:2 * D].astype(np.float32)
    Wv = W_qkv[:, 2 * D:3 * D].astype(np.float32)
    in_maps = []
    vs = []
    for b in range(B):
        xb = x[b].astype(np.float32)
        v = xb @ Wv                                  # [N, D] fp32 (exact-ish)
        vs.append(v)
        vpad = np.zeros((NT * P, D + 1), np.float32)
        vpad[0:SEQ, 0:D] = v
        vpad[0:SEQ, D] = 1.0
        va = np.ascontiguousarray(
            vpad.reshape(NT, P, D + 1).transpose(1, 0, 2)).astype(bf)
        qT = np.ascontiguousarray((xb @ Wq).T)       # [D, N], pre-scaled
        kT = np.ascontiguousarray((xb @ Wk).T)
        in_maps.append({
            "qT2": np.concatenate([qT, qT], axis=0).astype(bf),
            "kT2": np.concatenate([kT, kT], axis=0).astype(bf),
            "v_aug": va,
        })
    return in_maps, vs


def postprocess(results, vs, W_proj, b_proj):
    B = len(vs)
    out = np.empty((B, SEQ, D), np.float32)
    Wp = W_proj.astype(np.float32)
    bp = b_proj.astype(np.float32)
    for b in range(B):
        oT = results[b]["oT"]                        # [NCHUNK, 65, IC]
        O = oT.transpose(1, 0, 2).reshape(D + 1, NCHUNK * IC)[:, 0:SEQ]
        attn = (O[0:D] / O[D:D + 1]).T               # [N, D]
        out[b] = vs[b] + attn @ Wp + bp
    return out


def kernel(x, W_qkv, W_proj, b_proj):
    B = x.shape[0]
    if "nc" not in _cache:
        _cache["nc"] = build()
    nc = _cache["nc"]
    in_maps, vs = prep_in_maps(x, W_qkv, W_proj, b_proj)
    res = run_bass_kernel_spmd(nc, in_maps, core_ids=list(range(B)))
    return postprocess(res.results, vs, W_proj, b_proj)


if __name__ == "__main__":
    rng = np.random.default_rng(0)
    x = rng.standard_normal((8, SEQ, CH), dtype=np.float32)
    W_qkv = (rng.standard_normal((CH, 3 * D), dtype=np.float32) * CH ** -0.5)
    W_proj = (rng.standard_normal((D, D), dtype=np.float32) * D ** -0.5)
    b_proj = np.zeros(D, dtype=np.float32)
    out = kernel(x, W_qkv, W_proj, b_proj)
    print("out", out.shape, out.dtype)
